# revision 30
# baseline (speedup 1.0000x reference)
"""Trainium2 Bass kernel for additive (Bahdanau-style) attention with mask.

Reference computation (per batch b):
    d_proj = d @ W1.T + b1                         # [D]
    h_proj = h @ W2.T + b2                         # [T, D]
    o      = tanh(d_proj + h_proj)                 # [T, D]
    u      = o @ v                                 # [T]
    u      = where(mask, -1e9, u)
    a      = softmax(u)                            # [T]
    h'     = a @ h                                 # [D]
    return (h', a)

Full shapes: d[128,256] h[128,2048,256] mask[128,2048] -> h'[128,256], a[128,2048]

Sharding: pure data-parallel over batch B=128 across 8 NeuronCores
(16 batches per core); weights replicated.  No collectives.

Device algorithm per core (B_L=16, T=2048, D=256), f32 storage with
float32r matmuls (1 cyc/row at N>=256):
  * No softmax max-subtraction needed: |u| <= sum|v| ~= 13, exp is f32-safe.
    Mask handled multiplicatively on p = exp(u):  p *= (1-mask);  a = p/l.
  * Per (batch, 512-t tile): PE-transpose h -> hT [d_p, t]; matmul
    o_T[e_p, t] = W2T.T @ hT; ACT tanh with per-partition bias
    (d_proj+b2)[e]; matmul u[1, t] = vT.T @ o_T accumulated in PSUM [1,2048].
  * Per batch: ACT exp -> p[1,2048]; PE-transpose p into [128t, 16c];
    multiply by (1-mask)T; 16 rank-1 matmuls accumulate h' = sum p[t] h[t,:];
    l = column-sum reduce + final ones-matmul; scale by 1/l at the end.
"""

import numpy as np

import concourse.bass as bass
import concourse.mybir as mybir
from concourse.bass import ds, ts
from concourse.tile import TileContext
from concourse.masks import make_identity
from concourse.bass_utils import run_bass_kernel_spmd

P = 128          # SBUF partitions
BL = 16          # batches per core
T = 2048
D = 256
TT = 512         # t-tile size
NTILE = T // TT  # 4
NCHUNK = T // P  # 16 chunks of 128 t per batch
NCORES = 8

F32 = mybir.dt.float32
F32R = mybir.dt.float32r
BF16 = mybir.dt.bfloat16
U8 = mybir.dt.uint8
TANH = mybir.ActivationFunctionType.Tanh
EXP = mybir.ActivationFunctionType.Exp
AX = mybir.AxisListType
OP = mybir.AluOpType


def r(ap):
    """view a 4-byte f32 AP as float32r for TensorEngine ops"""
    return ap.bitcast(F32R)


def _split_multiwaits(nc):
    """This walrus build allows only ONE sync-wait per instruction (any
    opcode).  Tile emits multi-waits; hoist each extra wait onto its own
    same-engine InstNoOp inserted immediately before the instruction."""
    for f in nc.m.functions:
        for blk in f.blocks:
            new_insts = []
            for inst in blk.instructions:
                si = inst.sync_info
                n = len(si.on_wait) if si and si.on_wait else 0
                if n > 1:
                    for w in si.on_wait[:-1]:
                        new_insts.append(mybir.InstNoOp(
                            name=nc.get_next_instruction_name(),
                            ins=[], outs=[], engine=inst.engine,
                            sync_info=mybir.SyncInfo(on_wait=[w], on_update=[])))
                    si.on_wait = [si.on_wait[-1]]
                new_insts.append(inst)
            blk.instructions = new_insts


def build():
    nc = bass.Bass()

    d_p = nc.declare_dram_parameter("d", [BL, D], F32, isOutput=False)
    h_p = nc.declare_dram_parameter("h", [BL, T, D], F32, isOutput=False)
    m_p = nc.declare_dram_parameter("mask", [BL, T], U8, isOutput=False)
    w1_p = nc.declare_dram_parameter("W1", [D, D], F32, isOutput=False)
    b1_p = nc.declare_dram_parameter("b1", [D], F32, isOutput=False)
    w2_p = nc.declare_dram_parameter("W2", [D, D], F32, isOutput=False)
    b2_p = nc.declare_dram_parameter("b2", [D], F32, isOutput=False)
    v_p = nc.declare_dram_parameter("v", [D], F32, isOutput=False)
    hp_p = nc.declare_dram_parameter("h_prime", [BL, D], F32, isOutput=True)
    a_p = nc.declare_dram_parameter("a", [BL, T], F32, isOutput=True)

    with TileContext(nc) as tc:
        with (
            tc.tile_pool(name="const", bufs=1) as cp,
            tc.tile_pool(name="hbuf", bufs=3) as hp_pool,
            tc.tile_pool(name="work", bufs=3) as wp,
            tc.tile_pool(name="ps_ht", bufs=2, space="PSUM") as ps_ht,
            tc.tile_pool(name="ps_o", bufs=3, space="PSUM") as ps_o,
            tc.tile_pool(name="ps_u", bufs=2, space="PSUM") as ps_u,
            tc.tile_pool(name="ps_sm", bufs=1, space="PSUM") as ps_sm,
        ):
            # ---------------- setup ----------------
            ident = cp.tile([P, P], F32)
            make_identity(nc, ident)
            ident_r = cp.tile([P, P], F32R)
            nc.vector.tensor_copy(ident_r[:], ident[:])
            ident_bf = cp.tile([P, P], BF16)
            nc.vector.tensor_copy(ident_bf[:], ident[:])

            # natural-layout weight loads [128, 2ec, 256d]
            w1n = wp.tile([P, 2, D], F32, tag="wnat")
            nc.sync.dma_start(w1n[:], w1_p.rearrange("(c p) d -> p c d", p=P))
            w2n = wp.tile([P, 2, D], F32, tag="wnat")
            nc.sync.dma_start(w2n[:], w2_p.rearrange("(c p) d -> p c d", p=P))

            # transposed weights W*T[d_p, dc, e]
            w1t = cp.tile([P, 2, D], F32R)
            w2t = cp.tile([P, 2, D], BF16)
            for wn, wt in ((w1n, w1t), (w2n, w2t)):
                for ec in range(2):
                    for dc in range(2):
                        pst = ps_sm.tile([P, P], F32, tag="sm")
                        nc.tensor.transpose(pst[:], wn[:, ec, ds(dc * P, P)], ident[:])
                        nc.vector.tensor_copy(wt[:, dc, ts(ec, P)], pst[:])

            # b1, b2, v: load as 1-partition rows, PE-transpose to [e_p, 2ec];
            # the b1+b2 add happens after the copies (same-engine deps only)
            b1r = wp.tile([1, D], F32, tag="bias_a")
            nc.sync.dma_start(b1r[:], b1_p[:].unsqueeze(0))
            b2r = wp.tile([1, D], F32, tag="bias_b")
            nc.sync.dma_start(b2r[:], b2_p[:].unsqueeze(0))
            vrow = wp.tile([1, D], F32, tag="bias_c")
            nc.sync.dma_start(vrow[:], v_p[:].unsqueeze(0))
            bsum = cp.tile([P, 2], F32)
            b2col = wp.tile([P, 2], F32, tag="bias_d")
            vt = cp.tile([P, 2], BF16)
            for ec in range(2):
                psb = ps_sm.tile([P, 1], F32, tag="sm")
                nc.tensor.transpose(psb[:], b1r[0:1, ts(ec, P)], ident[:1, :1])
                nc.vector.tensor_copy(bsum[:, ec : ec + 1], psb[:])
                psb2 = ps_sm.tile([P, 1], F32, tag="sm")
                nc.tensor.transpose(psb2[:], b2r[0:1, ts(ec, P)], ident[:1, :1])
                nc.vector.tensor_copy(b2col[:, ec : ec + 1], psb2[:])
                psv = ps_sm.tile([P, 1], F32, tag="sm")
                nc.tensor.transpose(psv[:], vrow[0:1, ts(ec, P)], ident[:1, :1])
                nc.vector.tensor_copy(vt[:, ec : ec + 1], psv[:])
            nc.vector.tensor_add(bsum[:], bsum[:], b2col[:])

            # dT[d_p, dc, b] via transpose of d [16, 256]
            dnat = wp.tile([BL, D], F32, tag="dnat")
            nc.sync.dma_start(dnat[:], d_p[:])
            dt_ = cp.tile([P, 2, BL], F32R)
            for dc in range(2):
                pst = ps_sm.tile([P, BL], F32, tag="sm")
                nc.tensor.transpose(pst[:], dnat[:, ds(dc * P, P)], ident[:BL, :BL])
                nc.vector.tensor_copy(dt_[:, dc, :], pst[:])

            # d_projT[e_p, ec, b] = W1T.T @ dT + (b1+b2)
            dproj = cp.tile([P, 2, BL], F32)
            for ec in range(2):
                psd = ps_sm.tile([P, BL], F32, tag="sm")
                for dc in range(2):
                    nc.tensor.matmul(
                        psd[:], w1t[:, dc, ts(ec, P)], dt_[:, dc, :],
                        start=(dc == 0), stop=(dc == 1))
                nc.vector.tensor_tensor(
                    dproj[:, ec, :], psd[:],
                    bsum[:, ec, None].to_broadcast((P, BL)), OP.add)

            # mask -> (1-mask) f32, transposed to [128t, 16c, 16b]
            mnat = wp.tile([BL, T], U8, tag="mnat")
            nc.sync.dma_start(mnat[:], m_p[:])
            mf = wp.tile([BL, T], F32, tag="mf")
            nc.vector.tensor_scalar(mf[:], mnat[:], -1.0, 1.0, OP.mult, OP.add)
            mt = cp.tile([P, NCHUNK, BL], F32)
            for c in range(NCHUNK):
                psm = ps_sm.tile([P, BL], F32, tag="sm")
                nc.tensor.transpose(psm[:], mf[:, ts(c, P)], ident[:BL, :BL])
                nc.vector.tensor_copy(mt[:, c, :], psm[:])

            # persistent accumulators
            pa_all = cp.tile([P, NCHUNK, BL], BF16)   # masked p, all batches
            lcol = cp.tile([P, BL], F32)             # per-chunk partial l sums
            hp_un = cp.tile([1, BL, D], F32)         # unscaled h' (partition 0)
            ones_col = cp.tile([P, 1], F32)
            nc.gpsimd.memset(ones_col[:], 1.0)

            # ---------------- main loop ----------------
            # Software-pipelined emission: tile i's tanh/u/exp are emitted
            # after tile i+1's transposes+h_proj, and each batch's epilogue
            # after the next batch's first tile — so the PE never sits in a
            # wait right behind ACT/DVE latencies.
            def tile_front(b, i, hbf, hts, pu):
                """transposes + h_proj matmuls for (b, i).  hT comes from
                the batch-level XBAR DMA (hts) on even batches, or from PE
                identity matmuls on odd batches — splits the transpose load
                across the DMA engines and the PE."""
                psu = ps_u.tile([1, TT], F32, tag="u", name=f"psu_{b}_{i}")
                pso = [ps_o.tile([P, TT], F32, tag="o", name=f"pso{e}_{b}_{i}")
                       for e in range(2)]
                if hts is not None:
                    for dc in range(2):
                        rhs = hts[:, slice(8 * i + dc, 8 * i + 8, 2), :]
                        for ec in range(2):
                            nc.tensor.matmul(
                                pso[ec][:], w2t[:, dc, ts(ec, P)], rhs,
                                start=(dc == 0), stop=(dc == 1))
                    return (b, i, pso, psu, pu)
                htsb = wp.tile([P, 2, TT], BF16, tag="htsb", name=f"ht_{b}_{i}")
                for dc in range(2):
                    psht = ps_ht.tile([P, TT], F32, tag="ht", name=f"psht_{b}_{i}_{dc}")
                    for s in range(4):
                        nc.tensor.matmul(
                            psht[:, ts(s, P)],
                            hbf[:, 4 * i + s, ds(dc * P, P)],
                            ident_bf[:], start=True, stop=True)
                    nc.vector.tensor_copy(htsb[:, dc, :], psht[:])
                    for ec in range(2):
                        nc.tensor.matmul(
                            pso[ec][:], w2t[:, dc, ts(ec, P)], htsb[:, dc, :],
                            start=(dc == 0), stop=(dc == 1))
                return (b, i, pso, psu, pu)

            def tile_back(st):
                """tanh(+bias) ; u-matmuls ; exp for a completed tile"""
                b, i, pso, psu, pu = st
                for ec in range(2):
                    osb = wp.tile([P, TT], BF16, tag="osb", name=f"osb_{b}_{i}_{ec}")
                    nc.scalar.activation(
                        osb[:], pso[ec][:], TANH, bias=dproj[:, ec, b : b + 1])
                    nc.tensor.matmul(
                        psu[0:1, :], vt[:, ec : ec + 1], osb[:],
                        start=(ec == 0), stop=(ec == 1))
                nc.scalar.activation(pu[0:1, ts(i, TT)], psu[0:1, :], EXP)

            def epilogue(b, hbf, pu):
                # transpose p -> [128t, 16c], apply (1-mask), store to pa_all
                pspt = ps_sm.tile([P, NCHUNK], F32, tag="sm", name=f"pspt_{b}")
                for c in range(NCHUNK):
                    nc.tensor.transpose(
                        pspt[:, c : c + 1], pu[0:1, ts(c, P)], ident[:1, :1])
                nc.vector.tensor_tensor(
                    pa_all[:, :, b], pspt[:], mt[:, :, b], OP.mult)
                # l partial: column sums [128, 1] for this batch
                nc.vector.tensor_reduce(
                    lcol[:, b : b + 1], pa_all[:, :, b], AX.X, OP.add)
                # h' (unscaled) = sum_c  pT[:,c] . h[:,c,:]
                pshp = ps_sm.tile([1, D], F32, tag="sm", name=f"pshp_{b}")
                for c in range(NCHUNK):
                    nc.tensor.matmul(
                        pshp[:], pa_all[:, c, b : b + 1], hbf[:, c, :],
                        start=(c == 0), stop=(c == NCHUNK - 1))
                nc.vector.tensor_copy(hp_un[0:1, b, :], pshp[:])

            pending_tile = None      # (state from tile_front)
            pending_epi = None       # (b, hbf, pu)
            for b in range(BL):
                # whole-batch h load, cast f32->bf16 during SWDGE DMA,
                # split per 512-t tile for pipelining
                hbf = hp_pool.tile([P, NCHUNK, D], BF16, tag="hbf", name=f"hbf_{b}")
                nc.gpsimd.dma_start(
                    hbf[:], h_p[b].rearrange("(c p) d -> p c d", p=P))
                hts = None
                if b % 2 == 0:
                    hts = hp_pool.tile([P, 2 * NCHUNK, P], BF16, tag="hts",
                                       name=f"hts_{b}")
                    nc.sync.dma_start_transpose(
                        hts[:], hbf.rearrange("p c d -> p (c d)"))

                pu = wp.tile([1, T], F32, tag="pu", name=f"pu_{b}")
                for i in range(NTILE):
                    st = tile_front(b, i, hbf, hts, pu)
                    if pending_tile is not None:
                        tile_back(pending_tile)
                    pending_tile = st
                    if i == 1 and pending_epi is not None:
                        epilogue(*pending_epi)
                        pending_epi = None
                pending_epi = (b, hbf, pu)
            tile_back(pending_tile)
            epilogue(*pending_epi)

            # ---------------- finalization ----------------
            # l[b] = ones.T @ lcol  -> [BL, 1] psum (lhsT free dim = BL)
            psl = ps_sm.tile([BL, 1], F32, tag="sm")
            nc.tensor.matmul(psl[:], lcol[:], ones_col[:], start=True, stop=True)
            linv = cp.tile([BL, 1], F32)
            nc.vector.reciprocal(linv[:], psl[:])

            # linv to free-dim layout [1, BL]
            pslt = ps_sm.tile([1, BL], F32, tag="sm")
            nc.tensor.transpose(pslt[:], linv[:], ident[:BL, :BL])
            linvt = cp.tile([1, BL], F32)
            nc.vector.tensor_copy(linvt[:], pslt[:])

            # h_prime = hp_un * linv (broadcast along D), DMA flat
            hp_fin = cp.tile([1, BL, D], F32)
            nc.vector.tensor_tensor(
                hp_fin[:], hp_un[:],
                linvt[:, :, None].to_broadcast((1, BL, D)), OP.mult)
            nc.sync.dma_start(
                hp_p[:].rearrange("b d -> (b d)").unsqueeze(0),
                hp_fin.rearrange("p b d -> p (b d)"))

            # a: transpose pa_all back to [16b, 2048t], scale rows by 1/l, DMA
            anat = cp.tile([BL, T], F32)
            for c in range(NCHUNK):
                psa = ps_sm.tile([BL, P], BF16, tag="sm")
                nc.tensor.transpose(psa[:], pa_all[:, c, :], ident_bf[:])
                nc.vector.tensor_copy(anat[:, ts(c, P)], psa[:])
            nc.vector.tensor_scalar_mul(anat[:], anat[:], linv[:])
            nc.sync.dma_start(a_p[:], anat[:])

    _split_multiwaits(nc)
    return nc


_NC_CACHE = None


def _get_nc():
    global _NC_CACHE
    if _NC_CACHE is None:
        _NC_CACHE = build()
    return _NC_CACHE


def run(inputs, trace=False):
    nc = _get_nc()
    in_maps = []
    for i in range(NCORES):
        sl = slice(i * BL, (i + 1) * BL)
        in_maps.append({
            "d": np.ascontiguousarray(inputs["d"][sl], dtype=np.float32),
            "h": np.ascontiguousarray(inputs["h"][sl], dtype=np.float32),
            "mask": np.ascontiguousarray(inputs["mask"][sl]).view(np.uint8),
            "W1": np.asarray(inputs["W1"], dtype=np.float32),
            "b1": np.asarray(inputs["b1"], dtype=np.float32),
            "W2": np.asarray(inputs["W2"], dtype=np.float32),
            "b2": np.asarray(inputs["b2"], dtype=np.float32),
            "v": np.asarray(inputs["v"], dtype=np.float32),
        })
    res = run_bass_kernel_spmd(nc, in_maps, core_ids=list(range(NCORES)),
                               trace=trace)
    h_prime = np.concatenate([np.asarray(res.results[i]["h_prime"])
                              for i in range(NCORES)], axis=0)
    a = np.concatenate([np.asarray(res.results[i]["a"])
                        for i in range(NCORES)], axis=0)
    return (h_prime, a), res


def kernel(**inputs):
    out, _ = run(inputs, trace=False)
    return out


# revision 31
# speedup vs baseline: 1.2691x; 1.2691x over previous
"""Trainium2 Bass kernel for additive (Bahdanau-style) attention with mask.

Reference computation (per batch b):
    d_proj = d @ W1.T + b1                         # [D]
    h_proj = h @ W2.T + b2                         # [T, D]
    o      = tanh(d_proj + h_proj)                 # [T, D]
    u      = o @ v                                 # [T]
    u      = where(mask, -1e9, u)
    a      = softmax(u)                            # [T]
    h'     = a @ h                                 # [D]
    return (h', a)

Full shapes: d[128,256] h[128,2048,256] mask[128,2048] -> h'[128,256], a[128,2048]

Sharding: pure data-parallel over batch B=128 across 8 NeuronCores
(16 batches per core); weights replicated.  No collectives.

Device algorithm per core (B_L=16, T=2048, D=256), f32 storage with
float32r matmuls (1 cyc/row at N>=256):
  * No softmax max-subtraction needed: |u| <= sum|v| ~= 13, exp is f32-safe.
    Mask handled multiplicatively on p = exp(u):  p *= (1-mask);  a = p/l.
  * Per (batch, 512-t tile): PE-transpose h -> hT [d_p, t]; matmul
    o_T[e_p, t] = W2T.T @ hT; ACT tanh with per-partition bias
    (d_proj+b2)[e]; matmul u[1, t] = vT.T @ o_T accumulated in PSUM [1,2048].
  * Per batch: ACT exp -> p[1,2048]; PE-transpose p into [128t, 16c];
    multiply by (1-mask)T; 16 rank-1 matmuls accumulate h' = sum p[t] h[t,:];
    l = column-sum reduce + final ones-matmul; scale by 1/l at the end.
"""

import numpy as np

import concourse.bass as bass
import concourse.mybir as mybir
from concourse.bass import ds, ts
from concourse.tile import TileContext
from concourse.masks import make_identity
from concourse.bass_utils import run_bass_kernel_spmd

P = 128          # SBUF partitions
BL = 16          # batches per core
T = 2048
D = 256
TT = 512         # t-tile size
NTILE = T // TT  # 4
NCHUNK = T // P  # 16 chunks of 128 t per batch
NCORES = 8

F32 = mybir.dt.float32
F32R = mybir.dt.float32r
BF16 = mybir.dt.bfloat16
U8 = mybir.dt.uint8
TANH = mybir.ActivationFunctionType.Tanh
EXP = mybir.ActivationFunctionType.Exp
AX = mybir.AxisListType
OP = mybir.AluOpType


def r(ap):
    """view a 4-byte f32 AP as float32r for TensorEngine ops"""
    return ap.bitcast(F32R)


def _split_multiwaits(nc):
    """This walrus build allows only ONE sync-wait per instruction (any
    opcode).  Tile emits multi-waits; hoist each extra wait onto its own
    same-engine InstNoOp inserted immediately before the instruction."""
    for f in nc.m.functions:
        for blk in f.blocks:
            new_insts = []
            for inst in blk.instructions:
                si = inst.sync_info
                n = len(si.on_wait) if si and si.on_wait else 0
                if n > 1:
                    for w in si.on_wait[:-1]:
                        new_insts.append(mybir.InstNoOp(
                            name=nc.get_next_instruction_name(),
                            ins=[], outs=[], engine=inst.engine,
                            sync_info=mybir.SyncInfo(on_wait=[w], on_update=[])))
                    si.on_wait = [si.on_wait[-1]]
                new_insts.append(inst)
            blk.instructions = new_insts


def build():
    nc = bass.Bass()

    d_p = nc.declare_dram_parameter("d", [BL, D], F32, isOutput=False)
    h_p = nc.declare_dram_parameter("h", [BL, T, D], F32, isOutput=False)
    m_p = nc.declare_dram_parameter("mask", [BL, T], U8, isOutput=False)
    w1_p = nc.declare_dram_parameter("W1", [D, D], F32, isOutput=False)
    b1_p = nc.declare_dram_parameter("b1", [D], F32, isOutput=False)
    w2_p = nc.declare_dram_parameter("W2", [D, D], F32, isOutput=False)
    b2_p = nc.declare_dram_parameter("b2", [D], F32, isOutput=False)
    v_p = nc.declare_dram_parameter("v", [D], F32, isOutput=False)
    hp_p = nc.declare_dram_parameter("h_prime", [BL, D], F32, isOutput=True)
    a_p = nc.declare_dram_parameter("a", [BL, T], F32, isOutput=True)

    with TileContext(nc) as tc:
        with (
            tc.tile_pool(name="const", bufs=1) as cp,
            tc.tile_pool(name="hbuf", bufs=3) as hp_pool,
            tc.tile_pool(name="work", bufs=3) as wp,
            tc.tile_pool(name="ps_ht", bufs=2, space="PSUM") as ps_ht,
            tc.tile_pool(name="ps_o", bufs=3, space="PSUM") as ps_o,
            tc.tile_pool(name="ps_u", bufs=2, space="PSUM") as ps_u,
            tc.tile_pool(name="ps_sm", bufs=1, space="PSUM") as ps_sm,
        ):
            # ---------------- setup ----------------
            ident = cp.tile([P, P], F32)
            make_identity(nc, ident)
            ident_r = cp.tile([P, P], F32R)
            nc.vector.tensor_copy(ident_r[:], ident[:])
            ident_bf = cp.tile([P, P], BF16)
            nc.vector.tensor_copy(ident_bf[:], ident[:])

            # natural-layout weight loads [128, 2ec, 256d]
            w1n = wp.tile([P, 2, D], F32, tag="wnat")
            nc.sync.dma_start(w1n[:], w1_p.rearrange("(c p) d -> p c d", p=P))
            w2n = wp.tile([P, 2, D], F32, tag="wnat")
            nc.sync.dma_start(w2n[:], w2_p.rearrange("(c p) d -> p c d", p=P))

            # transposed weights W*T[d_p, dc, e]
            w1t = cp.tile([P, 2, D], F32R)
            w2t = cp.tile([P, 2, D], BF16)
            for wn, wt in ((w1n, w1t), (w2n, w2t)):
                for ec in range(2):
                    for dc in range(2):
                        pst = ps_sm.tile([P, P], F32, tag="sm")
                        nc.tensor.transpose(pst[:], wn[:, ec, ds(dc * P, P)], ident[:])
                        nc.vector.tensor_copy(wt[:, dc, ts(ec, P)], pst[:])

            # b1, b2, v: load as 1-partition rows, PE-transpose to [e_p, 2ec];
            # the b1+b2 add happens after the copies (same-engine deps only)
            b1r = wp.tile([1, D], F32, tag="bias_a")
            nc.sync.dma_start(b1r[:], b1_p[:].unsqueeze(0))
            b2r = wp.tile([1, D], F32, tag="bias_b")
            nc.sync.dma_start(b2r[:], b2_p[:].unsqueeze(0))
            vrow = wp.tile([1, D], F32, tag="bias_c")
            nc.sync.dma_start(vrow[:], v_p[:].unsqueeze(0))
            bsum = cp.tile([P, 2], F32)
            b2col = wp.tile([P, 2], F32, tag="bias_d")
            vt = cp.tile([P, 2], BF16)
            for ec in range(2):
                psb = ps_sm.tile([P, 1], F32, tag="sm")
                nc.tensor.transpose(psb[:], b1r[0:1, ts(ec, P)], ident[:1, :1])
                nc.vector.tensor_copy(bsum[:, ec : ec + 1], psb[:])
                psb2 = ps_sm.tile([P, 1], F32, tag="sm")
                nc.tensor.transpose(psb2[:], b2r[0:1, ts(ec, P)], ident[:1, :1])
                nc.vector.tensor_copy(b2col[:, ec : ec + 1], psb2[:])
                psv = ps_sm.tile([P, 1], F32, tag="sm")
                nc.tensor.transpose(psv[:], vrow[0:1, ts(ec, P)], ident[:1, :1])
                nc.vector.tensor_copy(vt[:, ec : ec + 1], psv[:])
            nc.vector.tensor_add(bsum[:], bsum[:], b2col[:])

            # dT[d_p, dc, b] via transpose of d [16, 256]
            dnat = wp.tile([BL, D], F32, tag="dnat")
            nc.sync.dma_start(dnat[:], d_p[:])
            dt_ = cp.tile([P, 2, BL], F32R)
            for dc in range(2):
                pst = ps_sm.tile([P, BL], F32, tag="sm")
                nc.tensor.transpose(pst[:], dnat[:, ds(dc * P, P)], ident[:BL, :BL])
                nc.vector.tensor_copy(dt_[:, dc, :], pst[:])

            # d_projT[e_p, ec, b] = W1T.T @ dT + (b1+b2)
            dproj = cp.tile([P, 2, BL], F32)
            for ec in range(2):
                psd = ps_sm.tile([P, BL], F32, tag="sm")
                for dc in range(2):
                    nc.tensor.matmul(
                        psd[:], w1t[:, dc, ts(ec, P)], dt_[:, dc, :],
                        start=(dc == 0), stop=(dc == 1))
                nc.vector.tensor_tensor(
                    dproj[:, ec, :], psd[:],
                    bsum[:, ec, None].to_broadcast((P, BL)), OP.add)

            # mask -> (1-mask) f32, transposed to [128t, 16c, 16b]
            mnat = wp.tile([BL, T], U8, tag="mnat")
            nc.sync.dma_start(mnat[:], m_p[:])
            mf = wp.tile([BL, T], F32, tag="mf")
            nc.vector.tensor_scalar(mf[:], mnat[:], -1.0, 1.0, OP.mult, OP.add)
            mt = cp.tile([P, NCHUNK, BL], F32)
            for c in range(NCHUNK):
                psm = ps_sm.tile([P, BL], F32, tag="sm")
                nc.tensor.transpose(psm[:], mf[:, ts(c, P)], ident[:BL, :BL])
                nc.vector.tensor_copy(mt[:, c, :], psm[:])

            # persistent accumulators
            pa_all = cp.tile([P, NCHUNK, BL], BF16)   # masked p, all batches
            lcol = cp.tile([P, BL], F32)             # per-chunk partial l sums
            hp_un = cp.tile([1, BL, D], F32)         # unscaled h' (partition 0)
            ones_col = cp.tile([P, 1], F32)
            nc.gpsimd.memset(ones_col[:], 1.0)

            # ---------------- main loop ----------------
            # Software-pipelined emission: tile i's tanh/u/exp are emitted
            # after tile i+1's transposes+h_proj, and each batch's epilogue
            # after the next batch's first tile — so the PE never sits in a
            # wait right behind ACT/DVE latencies.
            def transpose_batch(b, hbf):
                """materialize hT for the whole batch via PE identity
                matmuls: htb[128d, 2dc, 2048t] (segregated so the h_proj/u
                matmul stream stays dense and HAM-warm)"""
                htb = hp_pool.tile([P, 2, T], BF16, tag="htb", name=f"htb_{b}")
                for i in range(NTILE):
                    for dc in range(2):
                        psht = ps_ht.tile([P, TT], F32, tag="ht",
                                          name=f"psht_{b}_{i}_{dc}")
                        for s in range(4):
                            nc.tensor.matmul(
                                psht[:, ts(s, P)],
                                hbf[:, 4 * i + s, ds(dc * P, P)],
                                ident_bf[:], start=True, stop=True)
                        nc.vector.tensor_copy(
                            htb[:, dc, ts(i, TT)], psht[:])
                return htb

            def tile_front(b, i, htb, pu):
                """h_proj matmuls for (b, i) reading materialized hT"""
                psu = ps_u.tile([1, TT], F32, tag="u", name=f"psu_{b}_{i}")
                pso = [ps_o.tile([P, TT], F32, tag="o", name=f"pso{e}_{b}_{i}")
                       for e in range(2)]
                for dc in range(2):
                    for ec in range(2):
                        nc.tensor.matmul(
                            pso[ec][:], w2t[:, dc, ts(ec, P)],
                            htb[:, dc, ts(i, TT)],
                            start=(dc == 0), stop=(dc == 1))
                return (b, i, pso, psu, pu)

            def tile_back(st):
                """tanh(+bias) ; u-matmuls ; exp for a completed tile"""
                b, i, pso, psu, pu = st
                for ec in range(2):
                    osb = wp.tile([P, TT], BF16, tag="osb", name=f"osb_{b}_{i}_{ec}")
                    nc.scalar.activation(
                        osb[:], pso[ec][:], TANH, bias=dproj[:, ec, b : b + 1])
                    nc.tensor.matmul(
                        psu[0:1, :], vt[:, ec : ec + 1], osb[:],
                        start=(ec == 0), stop=(ec == 1))
                nc.scalar.activation(pu[0:1, ts(i, TT)], psu[0:1, :], EXP)

            def epilogue(b, hbf, pu):
                # transpose p -> [128t, 16c], apply (1-mask), store to pa_all
                pspt = ps_sm.tile([P, NCHUNK], F32, tag="sm", name=f"pspt_{b}")
                for c in range(NCHUNK):
                    nc.tensor.transpose(
                        pspt[:, c : c + 1], pu[0:1, ts(c, P)], ident[:1, :1])
                nc.vector.tensor_tensor(
                    pa_all[:, :, b], pspt[:], mt[:, :, b], OP.mult)
                # l partial: column sums [128, 1] for this batch
                nc.vector.tensor_reduce(
                    lcol[:, b : b + 1], pa_all[:, :, b], AX.X, OP.add)
                # h' (unscaled) = sum_c  pT[:,c] . h[:,c,:]
                pshp = ps_sm.tile([1, D], F32, tag="sm", name=f"pshp_{b}")
                for c in range(NCHUNK):
                    nc.tensor.matmul(
                        pshp[:], pa_all[:, c, b : b + 1], hbf[:, c, :],
                        start=(c == 0), stop=(c == NCHUNK - 1))
                nc.vector.tensor_copy(hp_un[0:1, b, :], pshp[:])

            pending_tile = None      # (state from tile_front)
            pending_epi = None       # (b, hbf, pu)
            for b in range(BL):
                # whole-batch h load, cast f32->bf16 during SWDGE DMA,
                # split per 512-t tile for pipelining
                hbf = hp_pool.tile([P, NCHUNK, D], BF16, tag="hbf", name=f"hbf_{b}")
                nc.gpsimd.dma_start(
                    hbf[:], h_p[b].rearrange("(c p) d -> p c d", p=P))
                htb = transpose_batch(b, hbf)

                pu = wp.tile([1, T], F32, tag="pu", name=f"pu_{b}")
                for i in range(NTILE):
                    st = tile_front(b, i, htb, pu)
                    if pending_tile is not None:
                        tile_back(pending_tile)
                    pending_tile = st
                    if i == 1 and pending_epi is not None:
                        epilogue(*pending_epi)
                        pending_epi = None
                pending_epi = (b, hbf, pu)
            tile_back(pending_tile)
            epilogue(*pending_epi)

            # ---------------- finalization ----------------
            # l[b] = ones.T @ lcol  -> [BL, 1] psum (lhsT free dim = BL)
            psl = ps_sm.tile([BL, 1], F32, tag="sm")
            nc.tensor.matmul(psl[:], lcol[:], ones_col[:], start=True, stop=True)
            linv = cp.tile([BL, 1], F32)
            nc.vector.reciprocal(linv[:], psl[:])

            # linv to free-dim layout [1, BL]
            pslt = ps_sm.tile([1, BL], F32, tag="sm")
            nc.tensor.transpose(pslt[:], linv[:], ident[:BL, :BL])
            linvt = cp.tile([1, BL], F32)
            nc.vector.tensor_copy(linvt[:], pslt[:])

            # h_prime = hp_un * linv (broadcast along D), DMA flat
            hp_fin = cp.tile([1, BL, D], F32)
            nc.vector.tensor_tensor(
                hp_fin[:], hp_un[:],
                linvt[:, :, None].to_broadcast((1, BL, D)), OP.mult)
            nc.sync.dma_start(
                hp_p[:].rearrange("b d -> (b d)").unsqueeze(0),
                hp_fin.rearrange("p b d -> p (b d)"))

            # a: transpose pa_all back to [16b, 2048t], scale rows by 1/l, DMA
            anat = cp.tile([BL, T], F32)
            for c in range(NCHUNK):
                psa = ps_sm.tile([BL, P], BF16, tag="sm")
                nc.tensor.transpose(psa[:], pa_all[:, c, :], ident_bf[:])
                nc.vector.tensor_copy(anat[:, ts(c, P)], psa[:])
            nc.vector.tensor_scalar_mul(anat[:], anat[:], linv[:])
            nc.sync.dma_start(a_p[:], anat[:])

    _split_multiwaits(nc)
    return nc


_NC_CACHE = None


def _get_nc():
    global _NC_CACHE
    if _NC_CACHE is None:
        _NC_CACHE = build()
    return _NC_CACHE


def run(inputs, trace=False):
    nc = _get_nc()
    in_maps = []
    for i in range(NCORES):
        sl = slice(i * BL, (i + 1) * BL)
        in_maps.append({
            "d": np.ascontiguousarray(inputs["d"][sl], dtype=np.float32),
            "h": np.ascontiguousarray(inputs["h"][sl], dtype=np.float32),
            "mask": np.ascontiguousarray(inputs["mask"][sl]).view(np.uint8),
            "W1": np.asarray(inputs["W1"], dtype=np.float32),
            "b1": np.asarray(inputs["b1"], dtype=np.float32),
            "W2": np.asarray(inputs["W2"], dtype=np.float32),
            "b2": np.asarray(inputs["b2"], dtype=np.float32),
            "v": np.asarray(inputs["v"], dtype=np.float32),
        })
    res = run_bass_kernel_spmd(nc, in_maps, core_ids=list(range(NCORES)),
                               trace=trace)
    h_prime = np.concatenate([np.asarray(res.results[i]["h_prime"])
                              for i in range(NCORES)], axis=0)
    a = np.concatenate([np.asarray(res.results[i]["a"])
                        for i in range(NCORES)], axis=0)
    return (h_prime, a), res


def kernel(**inputs):
    out, _ = run(inputs, trace=False)
    return out
